# revision 2
# baseline (speedup 1.0000x reference)
"""Trainium2 Bass kernel for single-head causal attention (v8).

Same math as v7 (transposed softmax S^T, WQ folded into the key side,
interleaved-key causal skip, host flash-combine) with a rebuilt schedule:

* PE warm-up block at t=0: dummy matmuls on a zero tile run during the
  initial DMA wait so the HAM clock-gate opens before real work arrives.
* Two DMA priority streams: weight/one-time loads go on the Activation
  (scalar) HWDGE queue, the x-streams + query-group prefetches on the SP
  (sync) queue, so neither blocks the other.  wk/wq/wv all load up front
  into simultaneously-live pools -> no phase-boundary DMA stalls.
* Query groups run ASCENDING (small causal groups first), so the final
  group is the compute-heaviest and hides its own prefetch + the output
  drain.
* The causal mask is group-invariant in this layout: one [128, 512]
  bf16 tile per core replaces the 8 MB maskT tensor.
* V path in bf16 (xvT, wv, ohat): halves that DMA traffic; scores stay
  fp32r (bf16 there fails the accuracy gate).
* Softmax denominator: DVE accumulates the exp slabs, one ones-matmul
  per group instead of one per k-tile.

Outputs: ohat [2048, 1024] bf16 (unnormalized), l [1, 2048] f32;
host combines out = (ohat0 + ohat1) / (l0 + l1).
"""

import ml_dtypes
import numpy as np

import concourse.bass as bass
from concourse import bacc
import concourse.mybir as mybir
import concourse.tile as tile
from concourse.bass_utils import run_bass_kernel_spmd

P = 128
B, S, DIN, DOUT = 4, 2048, 1024, 1024
KSH = S // 2        # key rows per core
KO = DIN // P       # 8 contraction sub-tiles
DO = DOUT // P      # 8 dout sub-tiles
NT = KSH // P       # 8 key tiles per core
QG = 512            # query-group width (psum free dim)
NG = S // QG        # 4 query groups
H = QG // 2
F32 = mybir.dt.float32
F32R = mybir.dt.float32r
BF16 = mybir.dt.bfloat16
SCALE = 1.0 / float(np.sqrt(DOUT))
NEG = -1.0e9
NWARM = 20          # PE warm-up matmuls
KCH = 256           # xk stream chunk (keeps K-phase SBUF peak in budget)
EXP = mybir.ActivationFunctionType.Exp

_NC_CACHE = {}


def _load_sliced(eng, dst, src, width, nslice):
    """DMA a [128, KO, width] tensor in slices for early availability."""
    step = width // nslice
    for s in range(nslice):
        eng.dma_start(
            out=dst[:, :, s * step : (s + 1) * step],
            in_=src[:, :, s * step : (s + 1) * step],
        )


def _emit_score_chunk(nc, psS, mfull_sb, kqt, xq_g, slab, zeros_sb, g, kt):
    """Scores+exp for one (group, k-tile) chunk of S^T.

    kqt is unscaled (KQ^T raw); 1/sqrt(d) is folded into the exp's scale.
    kt == 2g+1: first 256 query columns are fully masked for both cores
    (interleaved-key geometry) -> zero-fill, compute only the second half.
    kt == 2g: diagonal -> additive mask tile (group-invariant).
    """
    if kt == 2 * g + 1:
        ps = psS.tile([P, QG], F32, name="score_ps")
        ph = ps[:, H:]
        for io in range(KO):
            nc.tensor.matmul(
                ph, lhsT=kqt[:, io, kt * P : (kt + 1) * P],
                rhs=xq_g[:, io, H:],
                start=(io == 0), stop=(io == KO - 1),
            )
        nc.vector.tensor_copy(slab[:, kt, :H], zeros_sb[:, :H])
        nc.vector.tensor_tensor(
            slab[:, kt, H:], ph, mfull_sb[:, :H], mybir.AluOpType.add
        )
        nc.scalar.activation(slab[:, kt, H:], slab[:, kt, H:], EXP, scale=SCALE)
        return
    ps = psS.tile([P, QG], F32, name="score_ps")
    for io in range(KO):
        nc.tensor.matmul(
            ps, lhsT=kqt[:, io, kt * P : (kt + 1) * P], rhs=xq_g[:, io, :],
            start=(io == 0), stop=(io == KO - 1),
        )
    if kt == 2 * g:
        nc.vector.tensor_tensor(
            slab[:, kt, :], ps, mfull_sb, mybir.AluOpType.add
        )
        nc.scalar.activation(slab[:, kt, :], slab[:, kt, :], EXP, scale=SCALE)
    else:
        nc.scalar.activation(slab[:, kt, :], ps, EXP, scale=SCALE)


def _build_bass():
    nc = bacc.Bacc()
    xqT = nc.declare_dram_parameter("xqT", [DIN, S], F32R, isOutput=False)
    xkT = nc.declare_dram_parameter("xkT", [DIN, KSH], F32R, isOutput=False)
    xvT = nc.declare_dram_parameter("xvT", [DIN, KSH], BF16, isOutput=False)
    wqT = nc.declare_dram_parameter("wqT", [DOUT, DIN], F32R, isOutput=False)
    wk = nc.declare_dram_parameter("wk", [DIN, DOUT], F32R, isOutput=False)
    wv = nc.declare_dram_parameter("wv", [DIN, DOUT], BF16, isOutput=False)
    mfull = nc.declare_dram_parameter("mfull", [P, QG], BF16, isOutput=False)
    ohat = nc.declare_dram_parameter("ohat", [S, DOUT], BF16, isOutput=True)
    l_out = nc.declare_dram_parameter("l", [1, S], F32, isOutput=True)

    xq3 = xqT[:, :].rearrange("(o p) q -> p o q", p=P)
    xk3 = xkT[:, :].rearrange("(o p) s -> p o s", p=P)
    xv3 = xvT[:, :].rearrange("(o p) s -> p o s", p=P)
    wq3 = wqT[:, :].rearrange("(o p) i -> p o i", p=P)
    wk3 = wk[:, :].rearrange("(o p) d -> p o d", p=P)
    wv3 = wv[:, :].rearrange("(o p) d -> p o d", p=P)

    with tile.TileContext(nc) as tc:
        # ---- constants (zeros for warm-up + half-tile fill, ones for l,
        # the group-invariant causal mask)
        cpool_cm = tc.tile_pool(name="const", bufs=1)
        cpool = cpool_cm.__enter__()
        z32 = cpool.tile([P, QG], F32, name="z32")
        nc.vector.memset(z32, 0.0)
        zeros_sb = cpool.tile([P, QG], F32R, name="zeros")
        nc.vector.tensor_copy(zeros_sb, z32)
        ones32 = cpool.tile([P, 1], F32, name="ones32")
        nc.vector.memset(ones32, 1.0)
        ones_sb = cpool.tile([P, 1], F32R, name="ones")
        nc.vector.tensor_copy(ones_sb, ones32)
        mfull_sb = cpool.tile([P, QG], BF16, name="mfull_sb")

        # ---- PE warm-up: keep TensorE busy through the initial DMA wait
        # so HAM un-throttles before the first real matmul.
        with tc.tile_pool(name="ps_warm", bufs=1, space="PSUM") as psW:
            pw = psW.tile([P, QG], F32, name="warm_ps")
            for _ in range(NWARM):
                nc.tensor.matmul(
                    pw, lhsT=zeros_sb[:, :P], rhs=zeros_sb,
                    start=True, stop=True,
                )

        with tc.tile_pool(name="persist", bufs=1) as persist:
            v_sb = persist.tile([P, NT, DOUT], F32R, name="v")
            kqt_sb = persist.tile([P, KO, KSH], F32R, name="kqt")

            wvpool_cm = tc.tile_pool(name="wv_pool", bufs=1)
            wvpool = wvpool_cm.__enter__()
            wv_sb = wvpool.tile([P, KO, DOUT], BF16, name="wv_sb")

            wpool_cm = tc.tile_pool(name="w_pool", bufs=1)
            wpool = wpool_cm.__enter__()
            wk_sb = wpool.tile([P, KO, DOUT], F32R, name="wk_sb", tag="wk")
            wq_sb = wpool.tile([P, KO, DIN], F32R, name="wq_sb", tag="wq")

            ktpool_cm = tc.tile_pool(name="kt_pool", bufs=1)
            ktpool = ktpool_cm.__enter__()
            kt_sb = ktpool.tile([P, DO, KSH], F32R, name="kt")

            # One-time loads, scalar (Activation) HWDGE queue, priority order.
            _load_sliced(nc.scalar, wk_sb, wk3, DOUT, 8)
            _load_sliced(nc.scalar, wq_sb, wq3, DIN, 8)
            _load_sliced(nc.scalar, wv_sb, wv3, DOUT, 2)
            nc.scalar.dma_start(out=mfull_sb, in_=mfull[:, :])

            # ---- Phase K: K^T = WK^T @ Xk^T for this core's key half
            with (
                tc.tile_pool(name="xs_k", bufs=2) as xpool,
                tc.tile_pool(name="ps_k", bufs=4, space="PSUM") as psK,
            ):
                for c in range(KSH // KCH):
                    x_sb = xpool.tile([P, KO, KCH], F32R, name="xk_chunk")
                    for o in range(KO):
                        nc.sync.dma_start(
                            out=x_sb[:, o, :],
                            in_=xk3[:, o, c * KCH : (c + 1) * KCH],
                        )
                    for o in range(DO):
                        ps = psK.tile([P, KCH], F32, name="k_ps")
                        for k in range(KO):
                            nc.tensor.matmul(
                                ps,
                                lhsT=wk_sb[:, k, o * P : (o + 1) * P],
                                rhs=x_sb[:, k, :],
                                start=(k == 0),
                                stop=(k == KO - 1),
                            )
                        nc.vector.tensor_copy(
                            kt_sb[:, o, c * KCH : (c + 1) * KCH], ps
                        )

            # ---- Phase KQ: KQ^T = WQ @ K^T (unscaled; scale folded in exp)
            with tc.tile_pool(name="ps_kq", bufs=4, space="PSUM") as psKQ:
                for it in range(KO):
                    for kc in range(KSH // 512):
                        ps = psKQ.tile([P, 512], F32, name="kq_ps")
                        for do in range(DO):
                            nc.tensor.matmul(
                                ps,
                                lhsT=wq_sb[:, do, it * P : (it + 1) * P],
                                rhs=kt_sb[:, do, kc * 512 : (kc + 1) * 512],
                                start=(do == 0),
                                stop=(do == DO - 1),
                            )
                        nc.vector.tensor_copy(
                            kqt_sb[:, it, kc * 512 : (kc + 1) * 512], ps
                        )

            ktpool_cm.__exit__(None, None, None)
            wpool_cm.__exit__(None, None, None)

            # ---- post-KQ pools (live through attention)
            xvpool_cm = tc.tile_pool(name="xv_pool", bufs=2)
            xvpool = xvpool_cm.__enter__()
            xqpool_cm = tc.tile_pool(name="xq_pool", bufs=2)
            xqpool = xqpool_cm.__enter__()
            slabpool_cm = tc.tile_pool(name="slab", bufs=2)
            slabpool = slabpool_cm.__enter__()
            accpool_cm = tc.tile_pool(name="l_acc", bufs=2)
            accpool = accpool_cm.__enter__()
            lpool_cm = tc.tile_pool(name="l_row", bufs=2)
            lpool = lpool_cm.__enter__()
            opool_cm = tc.tile_pool(name="o_out", bufs=2)
            opool = opool_cm.__enter__()

            # xq prefetches: g0 on the scalar queue (behind the weights),
            # g1 on the sync queue (behind the xk stream).
            xq_tiles = {}
            xq_tiles[0] = xqpool.tile([P, KO, QG], F32R, name="xq_group")
            for o in range(KO):
                nc.scalar.dma_start(
                    out=xq_tiles[0][:, o, :], in_=xq3[:, o, 0:QG]
                )
            xq_tiles[1] = xqpool.tile([P, KO, QG], F32R, name="xq_group")
            for o in range(KO):
                nc.sync.dma_start(
                    out=xq_tiles[1][:, o, :], in_=xq3[:, o, QG : 2 * QG]
                )

            # ---- Phase V: V = Xv @ WV (bf16 inputs, fp32 accumulate)
            with tc.tile_pool(name="ps_v", bufs=4, space="PSUM") as psV:
                VCH = 512
                xv_tiles = []
                for c in range(KSH // VCH):
                    xv_sb = xvpool.tile([P, KO, VCH], BF16, name="xv_chunk")
                    for o in range(KO):
                        nc.sync.dma_start(
                            out=xv_sb[:, o, :],
                            in_=xv3[:, o, c * VCH : (c + 1) * VCH],
                        )
                    xv_tiles.append(xv_sb)
                for c in range(KSH // VCH):
                    for t in range(VCH // P):
                        for dh in range(DOUT // 512):
                            ps = psV.tile([P, 512], F32, name="v_ps")
                            for k in range(KO):
                                nc.tensor.matmul(
                                    ps,
                                    lhsT=xv_tiles[c][:, k, t * P : (t + 1) * P],
                                    rhs=wv_sb[:, k, dh * 512 : (dh + 1) * 512],
                                    start=(k == 0),
                                    stop=(k == KO - 1),
                                )
                            nc.vector.tensor_copy(
                                v_sb[
                                    :,
                                    c * (VCH // P) + t,
                                    dh * 512 : (dh + 1) * 512,
                                ],
                                ps,
                            )

            # ---- Attention: groups ascending; the last (largest) group
            # hides its own prefetch and the output drain.
            with (
                tc.tile_pool(name="ps_s", bufs=3, space="PSUM") as psS,
                tc.tile_pool(name="ps_l", bufs=2, space="PSUM") as psL,
                tc.tile_pool(name="ps_o", bufs=2, space="PSUM") as psO,
            ):
                for g in range(NG):
                    lim = min(NT, 2 * g + 2)
                    xq_g = xq_tiles[g]
                    slab = slabpool.tile([P, NT, QG], F32R, name="expT")
                    for kt in range(lim):
                        _emit_score_chunk(
                            nc, psS, mfull_sb, kqt_sb, xq_g,
                            slab, zeros_sb, g, kt,
                        )
                    # prefetch xq for g+2 (slot of g, now fully consumed)
                    if g + 2 < NG:
                        nxt = xqpool.tile([P, KO, QG], F32R, name="xq_group")
                        xq_tiles[g + 2] = nxt
                        for o in range(KO):
                            nc.sync.dma_start(
                                out=nxt[:, o, :],
                                in_=xq3[:, o, (g + 2) * QG : (g + 3) * QG],
                            )

                    # l: DVE-accumulate exp slabs, single ones-matmul
                    acc = accpool.tile([P, QG], F32R, name="acc")
                    nc.vector.tensor_tensor(
                        acc, slab[:, 0, :], slab[:, 1, :], mybir.AluOpType.add
                    )
                    for kt in range(2, lim):
                        nc.vector.tensor_tensor(
                            acc, acc, slab[:, kt, :], mybir.AluOpType.add
                        )
                    ps_l = psL.tile([1, QG], F32, name="l_ps")
                    nc.tensor.matmul(
                        ps_l, lhsT=ones_sb, rhs=acc, start=True, stop=True
                    )
                    l_t = lpool.tile([1, QG], F32, name="l_t")
                    nc.vector.tensor_copy(l_t, ps_l)
                    nc.scalar.dma_start(
                        out=l_out[:, g * QG : (g + 1) * QG], in_=l_t
                    )

                    # out: ohat^T-slice = slab^T @ V per 128-query tile
                    for t in range(QG // P):
                        kts = (
                            list(range(lim - 1))
                            if t < 2
                            else list(range(lim))
                        )
                        o_sb = opool.tile([P, DOUT], BF16, name="attn_out")
                        for dh in range(DOUT // 512):
                            ps = psO.tile([P, 512], F32, name="out_ps")
                            for kt in kts:
                                nc.tensor.matmul(
                                    ps,
                                    lhsT=slab[:, kt, t * P : (t + 1) * P],
                                    rhs=v_sb[:, kt, dh * 512 : (dh + 1) * 512],
                                    start=(kt == kts[0]),
                                    stop=(kt == kts[-1]),
                                )
                            nc.scalar.copy(
                                o_sb[:, dh * 512 : (dh + 1) * 512], ps
                            )
                        q0 = g * QG + t * P
                        nc.scalar.dma_start(out=ohat[q0 : q0 + P, :], in_=o_sb)

            opool_cm.__exit__(None, None, None)
            lpool_cm.__exit__(None, None, None)
            accpool_cm.__exit__(None, None, None)
            slabpool_cm.__exit__(None, None, None)
            xqpool_cm.__exit__(None, None, None)
            xvpool_cm.__exit__(None, None, None)
            wvpool_cm.__exit__(None, None, None)
        cpool_cm.__exit__(None, None, None)
    nc.finalize()
    return nc


def _get_nc():
    if "nc" not in _NC_CACHE:
        _NC_CACHE["nc"] = _build_bass()
    return _NC_CACHE["nc"]


def _key_index(hk):
    """Global key rows owned by core hk: interleaved 128-row blocks."""
    blocks = np.arange(hk, S // P, 2)
    return (blocks[:, None] * P + np.arange(P)[None, :]).reshape(-1)


def _mask_full(hk):
    """Group-invariant additive causal mask for the diagonal chunks.

    In the interleaved layout, chunk kt == 2g covers global keys
    (4g+hk)*128 + p against queries 512g + j, so the causal condition
    k > q reduces to hk*128 + p > j for every group; chunk kt == 2g+1's
    live half reduces to the same tile's first 256 columns.
    """
    p = np.arange(P)[:, None]
    j = np.arange(QG)[None, :]
    m = np.where(hk * P + p > j, np.float32(NEG), np.float32(0.0))
    return np.ascontiguousarray(m.astype(ml_dtypes.bfloat16))


def kernel(
    inputs_for_keys,
    inputs_for_values,
    inputs_for_queries,
    WK,
    WV,
    WQ,
    _trace=False,
):
    xk = np.asarray(inputs_for_keys, dtype=np.float32)
    xv = np.asarray(inputs_for_values, dtype=np.float32)
    xq = np.asarray(inputs_for_queries, dtype=np.float32)
    wk_ = np.ascontiguousarray(np.asarray(WK, dtype=np.float32))
    wv_ = np.ascontiguousarray(
        np.asarray(WV, dtype=np.float32).astype(ml_dtypes.bfloat16)
    )
    wqT_ = np.ascontiguousarray(np.asarray(WQ, dtype=np.float32).T)

    kidx = {hk: _key_index(hk) for hk in (0, 1)}
    masks = {hk: _mask_full(hk) for hk in (0, 1)}
    xqTb = [np.ascontiguousarray(xq[b].T) for b in range(B)]

    in_maps = []
    for i in range(8):
        b, hk = i // 2, i % 2
        in_maps.append(
            {
                "xqT": xqTb[b],
                "xkT": np.ascontiguousarray(xk[b][kidx[hk]].T),
                "xvT": np.ascontiguousarray(
                    xv[b][kidx[hk]].T.astype(ml_dtypes.bfloat16)
                ),
                "wqT": wqT_,
                "wk": wk_,
                "wv": wv_,
                "mfull": masks[hk],
            }
        )

    nc = _get_nc()
    res = run_bass_kernel_spmd(nc, in_maps, list(range(8)), trace=_trace)

    out = np.empty((B, S, DOUT), dtype=np.float32)
    for b in range(B):
        r0 = res.results[2 * b]
        r1 = res.results[2 * b + 1]
        den = (r0["l"] + r1["l"]).reshape(S, 1)
        out[b] = (
            r0["ohat"].astype(np.float32) + r1["ohat"].astype(np.float32)
        ) / den
    if _trace:
        return out, res
    return out


# revision 7
# speedup vs baseline: 1.0948x; 1.0948x over previous
"""Trainium2 Bass kernel for single-head causal attention (v8.2).

v7's math (transposed softmax S^T, WQ folded into the key side,
interleaved-key causal skip, host flash-combine) with a rebuilt schedule:

* PE warm-up block at t=0 (HAM un-throttles during the initial DMA wait).
* Two HWDGE priority streams. scalar (Activation) queue: wk -> wq ->
  mask -> (post-K) wv -> (post-KQ) xq_g0 -> per-group l/ohat writes.
  sync (SP) queue: xk stream -> (post-K) xv -> (post-KQ) xq_g1 ->
  in-loop xq prefetches.  Triggers are emitted at the program point
  where they should fire: a trigger only runs when the issuing engine's
  instruction stream reaches it.
* wv loads into wk's pool slot (tag reuse): its DMA fires exactly when
  the K phase retires, off the early critical window.
* One PSUM pool spans K/KQ/V (tag rotation, no pool-boundary syncs).
* Query groups ascend so the last (largest) group hides its own
  prefetch and the output drain; ohat is written per 512-col half.
* Causal mask is group-invariant: one [128, 512] bf16 tile per core.
* V path in bf16 (xvT, wv, ohat); score path stays fp32r (bf16 there
  fails the accuracy gate).  1/sqrt(d) folded into the exp scale.
* Softmax denominator: DVE accumulates exp slabs; one ones-matmul per
  group.

Outputs: ohat [2048, 1024] bf16 (unnormalized), l [1, 2048] f32;
host combines out = (ohat0 + ohat1) / (l0 + l1).
"""

import ml_dtypes
import numpy as np

import concourse.bass as bass
from concourse import bacc
import concourse.mybir as mybir
import concourse.tile as tile
from concourse.bass_utils import run_bass_kernel_spmd

P = 128
B, S, DIN, DOUT = 4, 2048, 1024, 1024
KSH = S // 2        # key rows per core
KO = DIN // P       # 8 contraction sub-tiles
DO = DOUT // P      # 8 dout sub-tiles
NT = KSH // P       # 8 key tiles per core
QG = 512            # query-group width (psum free dim)
NG = S // QG        # 4 query groups
H = QG // 2
F32 = mybir.dt.float32
F32R = mybir.dt.float32r
BF16 = mybir.dt.bfloat16
SCALE = 1.0 / float(np.sqrt(DOUT))
NEG = -1.0e9
NWARM = 20          # PE warm-up matmuls
KCH = 256           # xk stream chunk
EXP = mybir.ActivationFunctionType.Exp

_NC_CACHE = {}


def _load_sliced(eng, dst, src, width, nslice):
    step = width // nslice
    for s in range(nslice):
        eng.dma_start(
            out=dst[:, :, s * step : (s + 1) * step],
            in_=src[:, :, s * step : (s + 1) * step],
        )


def _emit_score_chunk(nc, psS, mfull_sb, kqt, xq_g, slab, zeros_sb, g, kt):
    """Scores+exp for one (group, k-tile) chunk of S^T.

    kqt is unscaled; 1/sqrt(d) is folded into the exp's scale (the
    additive mask just scales along, -1e9/32 still floors the exp).
    kt == 2g+1: first 256 query columns fully masked -> zero-fill,
    compute the second half only.  kt == 2g: diagonal, additive mask.
    """
    if kt == 2 * g + 1:
        ps = psS.tile([P, QG], F32, name="score_ps")
        ph = ps[:, H:]
        for io in range(KO):
            nc.tensor.matmul(
                ph, lhsT=kqt[:, io, kt * P : (kt + 1) * P],
                rhs=xq_g[:, io, H:],
                start=(io == 0), stop=(io == KO - 1),
            )
        nc.vector.tensor_copy(slab[:, kt, :H], zeros_sb[:, :H])
        nc.vector.tensor_tensor(
            slab[:, kt, H:], ph, mfull_sb[:, :H], mybir.AluOpType.add
        )
        nc.scalar.activation(slab[:, kt, H:], slab[:, kt, H:], EXP, scale=SCALE)
        return
    ps = psS.tile([P, QG], F32, name="score_ps")
    for io in range(KO):
        nc.tensor.matmul(
            ps, lhsT=kqt[:, io, kt * P : (kt + 1) * P], rhs=xq_g[:, io, :],
            start=(io == 0), stop=(io == KO - 1),
        )
    if kt == 2 * g:
        nc.vector.tensor_tensor(
            slab[:, kt, :], ps, mfull_sb, mybir.AluOpType.add
        )
        nc.scalar.activation(slab[:, kt, :], slab[:, kt, :], EXP, scale=SCALE)
    else:
        nc.scalar.activation(slab[:, kt, :], ps, EXP, scale=SCALE)


def _build_bass():
    nc = bacc.Bacc()
    xqT = nc.declare_dram_parameter("xqT", [DIN, S], F32R, isOutput=False)
    xkT = nc.declare_dram_parameter("xkT", [DIN, KSH], F32R, isOutput=False)
    xvT = nc.declare_dram_parameter("xvT", [DIN, KSH], BF16, isOutput=False)
    wqT = nc.declare_dram_parameter("wqT", [DOUT, DIN], F32R, isOutput=False)
    wk = nc.declare_dram_parameter("wk", [DIN, DOUT], F32R, isOutput=False)
    wv = nc.declare_dram_parameter("wv", [DIN, DOUT], BF16, isOutput=False)
    mfull = nc.declare_dram_parameter("mfull", [P, QG], BF16, isOutput=False)
    ohat = nc.declare_dram_parameter("ohat", [S, DOUT], BF16, isOutput=True)
    l_out = nc.declare_dram_parameter("l", [1, S], F32, isOutput=True)

    xq3 = xqT[:, :].rearrange("(o p) q -> p o q", p=P)
    xk3 = xkT[:, :].rearrange("(o p) s -> p o s", p=P)
    xv3 = xvT[:, :].rearrange("(o p) s -> p o s", p=P)
    wq3 = wqT[:, :].rearrange("(o p) i -> p o i", p=P)
    wk3 = wk[:, :].rearrange("(o p) d -> p o d", p=P)
    wv3 = wv[:, :].rearrange("(o p) d -> p o d", p=P)

    with tile.TileContext(nc) as tc:
        # ---- constants
        cpool_cm = tc.tile_pool(name="const", bufs=1)
        cpool = cpool_cm.__enter__()
        z32 = cpool.tile([P, QG], F32, name="z32")
        nc.vector.memset(z32, 0.0)
        zeros_sb = cpool.tile([P, QG], F32R, name="zeros")
        nc.vector.tensor_copy(zeros_sb, z32)
        ones32 = cpool.tile([P, 1], F32, name="ones32")
        nc.vector.memset(ones32, 1.0)
        ones_sb = cpool.tile([P, 1], F32R, name="ones")
        nc.vector.tensor_copy(ones_sb, ones32)
        mfull_sb = cpool.tile([P, QG], BF16, name="mfull_sb")

        # ---- PE warm-up through the initial DMA wait
        with tc.tile_pool(name="ps_warm", bufs=1, space="PSUM") as psW:
            pw = psW.tile([P, QG], F32, name="warm_ps")
            for _ in range(NWARM):
                nc.tensor.matmul(
                    pw, lhsT=zeros_sb[:, :P], rhs=zeros_sb,
                    start=True, stop=True,
                )

        with tc.tile_pool(name="persist", bufs=1) as persist:
            v_sb = persist.tile([P, NT, DOUT], F32R, name="v")
            kqt_sb = persist.tile([P, KO, KSH], F32R, name="kqt")

            # wk now; wv reuses this slot after the K phase retires
            wpool_cm = tc.tile_pool(name="w_pool", bufs=1)
            wpool = wpool_cm.__enter__()
            wk_sb = wpool.tile([P, KO, DOUT], F32R, name="wk_sb", tag="w")

            xvpool_cm = tc.tile_pool(name="xv_pool", bufs=2)
            xvpool = xvpool_cm.__enter__()

            wqpool_cm = tc.tile_pool(name="wq_pool", bufs=1)
            wqpool = wqpool_cm.__enter__()
            wq_sb = wqpool.tile([P, KO, DIN], F32R, name="wq_sb")

            ktpool_cm = tc.tile_pool(name="kt_pool", bufs=1)
            ktpool = ktpool_cm.__enter__()
            kt_sb = ktpool.tile([P, DO, KSH], F32R, name="kt")

            # early one-time loads (scalar HWDGE queue, priority order)
            _load_sliced(nc.scalar, wk_sb, wk3, DOUT, 8)
            _load_sliced(nc.scalar, wq_sb, wq3, DIN, 8)
            nc.scalar.dma_start(out=mfull_sb, in_=mfull[:, :])

            # one PSUM pool spans K/KQ/V: tag rotation, no boundary syncs
            pspool_cm = tc.tile_pool(name="ps_main", bufs=4, space="PSUM")
            pspool = pspool_cm.__enter__()

            # ---- Phase K: K^T = WK^T @ Xk^T
            with tc.tile_pool(name="xs_k", bufs=3) as xpool:
                for c in range(KSH // KCH):
                    x_sb = xpool.tile([P, KO, KCH], F32R, name="xk_chunk")
                    for o in range(KO):
                        nc.sync.dma_start(
                            out=x_sb[:, o, :],
                            in_=xk3[:, o, c * KCH : (c + 1) * KCH],
                        )
                    for o in range(DO):
                        ps = pspool.tile([P, QG], F32, name="mm_ps")
                        psn = ps[:, :KCH]
                        for k in range(KO):
                            nc.tensor.matmul(
                                psn,
                                lhsT=wk_sb[:, k, o * P : (o + 1) * P],
                                rhs=x_sb[:, k, :],
                                start=(k == 0),
                                stop=(k == KO - 1),
                            )
                        nc.vector.tensor_copy(
                            kt_sb[:, o, c * KCH : (c + 1) * KCH], psn
                        )

            # post-K loads: wv into wk's slot (fires when K retires),
            # xv behind the xk stream on sync.
            wv_sb = wpool.tile([P, KO, DOUT], BF16, name="wv_sb", tag="w")
            _load_sliced(nc.scalar, wv_sb, wv3, DOUT, 2)
            xv_tiles = []
            for c in range(2):
                xv_sb = xvpool.tile([P, KO, KSH // 2], BF16, name="xv_chunk")
                for o in range(KO):
                    nc.sync.dma_start(
                        out=xv_sb[:, o, :],
                        in_=xv3[:, o, c * (KSH // 2) : (c + 1) * (KSH // 2)],
                    )
                xv_tiles.append(xv_sb)

            # ---- Phase KQ: KQ^T = WQ @ K^T (unscaled)
            for it in range(KO):
                for kc in range(KSH // 512):
                    ps = pspool.tile([P, QG], F32, name="mm_ps")
                    for do in range(DO):
                        nc.tensor.matmul(
                            ps,
                            lhsT=wq_sb[:, do, it * P : (it + 1) * P],
                            rhs=kt_sb[:, do, kc * 512 : (kc + 1) * 512],
                            start=(do == 0),
                            stop=(do == DO - 1),
                        )
                    nc.vector.tensor_copy(
                        kqt_sb[:, it, kc * 512 : (kc + 1) * 512], ps
                    )

            ktpool_cm.__exit__(None, None, None)
            wqpool_cm.__exit__(None, None, None)

            # ---- attention pools + early query-group prefetches
            xqpool_cm = tc.tile_pool(name="xq_pool", bufs=2)
            xqpool = xqpool_cm.__enter__()
            slabpool_cm = tc.tile_pool(name="slab", bufs=2)
            slabpool = slabpool_cm.__enter__()
            accpool_cm = tc.tile_pool(name="l_acc", bufs=2)
            accpool = accpool_cm.__enter__()
            lpool_cm = tc.tile_pool(name="l_row", bufs=2)
            lpool = lpool_cm.__enter__()
            opool_cm = tc.tile_pool(name="o_out", bufs=3)
            opool = opool_cm.__enter__()

            xq_tiles = {}
            xq_tiles[0] = xqpool.tile([P, KO, QG], F32R, name="xq_group")
            for o in range(KO):
                nc.scalar.dma_start(
                    out=xq_tiles[0][:, o, :], in_=xq3[:, o, 0:QG]
                )
            xq_tiles[1] = xqpool.tile([P, KO, QG], F32R, name="xq_group")
            for o in range(KO):
                nc.sync.dma_start(
                    out=xq_tiles[1][:, o, :], in_=xq3[:, o, QG : 2 * QG]
                )

            # ---- Phase V: V = Xv @ WV (bf16 in, fp32 accum)
            for c in range(2):
                for t in range(KSH // 2 // P):
                    for dh in range(DOUT // 512):
                        ps = pspool.tile([P, QG], F32, name="mm_ps")
                        for k in range(KO):
                            nc.tensor.matmul(
                                ps,
                                lhsT=xv_tiles[c][:, k, t * P : (t + 1) * P],
                                rhs=wv_sb[:, k, dh * 512 : (dh + 1) * 512],
                                start=(k == 0),
                                stop=(k == KO - 1),
                            )
                        nc.vector.tensor_copy(
                            v_sb[
                                :,
                                c * (KSH // 2 // P) + t,
                                dh * 512 : (dh + 1) * 512,
                            ],
                            ps,
                        )

            pspool_cm.__exit__(None, None, None)

            # ---- Attention, groups ascending
            with (
                tc.tile_pool(name="ps_s", bufs=3, space="PSUM") as psS,
                tc.tile_pool(name="ps_l", bufs=1, space="PSUM") as psL,
                tc.tile_pool(name="ps_o", bufs=3, space="PSUM") as psO,
            ):
                for g in range(NG):
                    lim = min(NT, 2 * g + 2)
                    xq_g = xq_tiles[g]
                    slab = slabpool.tile([P, NT, QG], F32R, name="expT")
                    for kt in range(lim):
                        _emit_score_chunk(
                            nc, psS, mfull_sb, kqt_sb, xq_g,
                            slab, zeros_sb, g, kt,
                        )
                    # prefetch xq for g+2 (slot of g, scores done above)
                    if g + 2 < NG:
                        nxt = xqpool.tile([P, KO, QG], F32R, name="xq_group")
                        xq_tiles[g + 2] = nxt
                        for o in range(KO):
                            nc.sync.dma_start(
                                out=nxt[:, o, :],
                                in_=xq3[:, o, (g + 2) * QG : (g + 3) * QG],
                            )

                    # l: DVE-accumulate exp slabs, one ones-matmul
                    acc = accpool.tile([P, QG], F32R, name="acc")
                    nc.vector.tensor_tensor(
                        acc, slab[:, 0, :], slab[:, 1, :], mybir.AluOpType.add
                    )
                    for kt in range(2, lim):
                        nc.vector.tensor_tensor(
                            acc, acc, slab[:, kt, :], mybir.AluOpType.add
                        )
                    ps_l = psL.tile([1, QG], F32, name="l_ps")
                    nc.tensor.matmul(
                        ps_l, lhsT=ones_sb, rhs=acc, start=True, stop=True
                    )
                    l_t = lpool.tile([1, QG], F32, name="l_t")
                    nc.vector.tensor_copy(l_t, ps_l)
                    nc.scalar.dma_start(
                        out=l_out[:, g * QG : (g + 1) * QG], in_=l_t
                    )

                    # out: ohat slice = slab^T @ V; the very last tile is
                    # written per dh half so its DMA starts earlier
                    for t in range(QG // P):
                        kts = (
                            list(range(lim - 1))
                            if t < 2
                            else list(range(lim))
                        )
                        q0 = g * QG + t * P
                        last_tile = g == NG - 1 and t == QG // P - 1
                        o_sb = opool.tile([P, DOUT], BF16, name="attn_out")
                        for dh in range(DOUT // 512):
                            ps = psO.tile([P, 512], F32, name="out_ps")
                            for kt in kts:
                                nc.tensor.matmul(
                                    ps,
                                    lhsT=slab[:, kt, t * P : (t + 1) * P],
                                    rhs=v_sb[:, kt, dh * 512 : (dh + 1) * 512],
                                    start=(kt == kts[0]),
                                    stop=(kt == kts[-1]),
                                )
                            nc.scalar.copy(
                                o_sb[:, dh * 512 : (dh + 1) * 512], ps
                            )
                            if last_tile:
                                nc.sync.dma_start(
                                    out=ohat[
                                        q0 : q0 + P, dh * 512 : (dh + 1) * 512
                                    ],
                                    in_=o_sb[:, dh * 512 : (dh + 1) * 512],
                                )
                        if not last_tile:
                            nc.sync.dma_start(out=ohat[q0 : q0 + P, :], in_=o_sb)

            opool_cm.__exit__(None, None, None)
            lpool_cm.__exit__(None, None, None)
            accpool_cm.__exit__(None, None, None)
            slabpool_cm.__exit__(None, None, None)
            xqpool_cm.__exit__(None, None, None)
            xvpool_cm.__exit__(None, None, None)
            wpool_cm.__exit__(None, None, None)
        cpool_cm.__exit__(None, None, None)
    nc.finalize()
    return nc


def _get_nc():
    if "nc" not in _NC_CACHE:
        _NC_CACHE["nc"] = _build_bass()
    return _NC_CACHE["nc"]


def _key_index(hk):
    """Global key rows owned by core hk: interleaved 128-row blocks."""
    blocks = np.arange(hk, S // P, 2)
    return (blocks[:, None] * P + np.arange(P)[None, :]).reshape(-1)


def _mask_full(hk):
    """Group-invariant additive causal mask for the diagonal chunks.

    Chunk kt == 2g covers global keys (4g+hk)*128 + p against queries
    512g + j: causal k > q reduces to hk*128 + p > j for every g; chunk
    kt == 2g+1's live half reduces to this tile's first 256 columns.
    """
    p = np.arange(P)[:, None]
    j = np.arange(QG)[None, :]
    m = np.where(hk * P + p > j, np.float32(NEG), np.float32(0.0))
    return np.ascontiguousarray(m.astype(ml_dtypes.bfloat16))


def kernel(
    inputs_for_keys,
    inputs_for_values,
    inputs_for_queries,
    WK,
    WV,
    WQ,
    _trace=False,
):
    xk = np.asarray(inputs_for_keys, dtype=np.float32)
    xv = np.asarray(inputs_for_values, dtype=np.float32)
    xq = np.asarray(inputs_for_queries, dtype=np.float32)
    wk_ = np.ascontiguousarray(np.asarray(WK, dtype=np.float32))
    wv_ = np.ascontiguousarray(
        np.asarray(WV, dtype=np.float32).astype(ml_dtypes.bfloat16)
    )
    wqT_ = np.ascontiguousarray(np.asarray(WQ, dtype=np.float32).T)

    kidx = {hk: _key_index(hk) for hk in (0, 1)}
    masks = {hk: _mask_full(hk) for hk in (0, 1)}
    xqTb = [np.ascontiguousarray(xq[b].T) for b in range(B)]

    in_maps = []
    for i in range(8):
        b, hk = i // 2, i % 2
        in_maps.append(
            {
                "xqT": xqTb[b],
                "xkT": np.ascontiguousarray(xk[b][kidx[hk]].T),
                "xvT": np.ascontiguousarray(
                    xv[b][kidx[hk]].T.astype(ml_dtypes.bfloat16)
                ),
                "wqT": wqT_,
                "wk": wk_,
                "wv": wv_,
                "mfull": masks[hk],
            }
        )

    nc = _get_nc()
    res = run_bass_kernel_spmd(nc, in_maps, list(range(8)), trace=_trace)

    out = np.empty((B, S, DOUT), dtype=np.float32)
    for b in range(B):
        r0 = res.results[2 * b]
        r1 = res.results[2 * b + 1]
        den = (r0["l"] + r1["l"]).reshape(S, 1)
        out[b] = (
            r0["ohat"].astype(np.float32) + r1["ohat"].astype(np.float32)
        ) / den
    if _trace:
        return out, res
    return out


# revision 8
# speedup vs baseline: 1.1446x; 1.0454x over previous
"""Trainium2 Bass kernel for single-head causal attention (v8.4).

Math as v7 (transposed softmax S^T, WQ folded into the key side,
interleaved-key causal skip, host flash-combine); schedule rebuilt:

* PE warm-up block at t=0 (HAM un-throttles during the initial DMA wait).
* All streamed tensors are repacked on the host into [128, ...] layouts
  whose DMA slices are fully contiguous per partition row (4-16KB
  lines, one descriptor per transfer) — the v8.2 profile showed the
  1KB-line strided loads starving the K phase.
* Two HWDGE queues; triggers are emitted at the program point where
  they should fire.  scalar: wk -> wq(streamed it-slices) -> (post-K)
  wv -> (post-KQ) xq_g0 -> l rows.  sync: xk stream -> (post-K) xv ->
  (post-KQ) xq_g1 -> in-loop xq prefetches + ohat writes.
* wv loads into wk's pool slot (tag reuse) so its DMA fires exactly at
  K-phase retire; wq streams through 3 rotating 4KB it-slices, which
  frees enough SBUF for 512-wide xk chunks (N=512 matmuls).
* One PSUM pool spans K/KQ/V (tag rotation, no boundary syncs).
* Attention is software-pipelined ascending: S0 S1 l0 O0 S2 l1 O1 S3
  l2 O2 l3 O3 — exp/DVE latency of each group hides under the next
  group's score matmuls, and the big last group hides the drain.
* Causal mask is group-invariant: one [128, 512] bf16 tile per core.
* V path bf16 (xv, wv, ohat); score path fp32r.  1/sqrt(d) folded
  into the exp scale; softmax denominator via DVE accumulation + one
  ones-matmul per group.

Outputs: ohat [2048, 1024] bf16 (unnormalized), l [1, 2048] f32;
host combines out = (ohat0 + ohat1) / (l0 + l1).
"""

import ml_dtypes
import numpy as np

import concourse.bass as bass
from concourse import bacc
import concourse.mybir as mybir
import concourse.tile as tile
from concourse.bass_utils import run_bass_kernel_spmd

P = 128
B, S, DIN, DOUT = 4, 2048, 1024, 1024
KSH = S // 2        # key rows per core
KO = DIN // P       # 8 contraction sub-tiles
DO = DOUT // P      # 8 dout sub-tiles
NT = KSH // P       # 8 key tiles per core
QG = 512            # query-group width (psum free dim)
NG = S // QG        # 4 query groups
H = QG // 2
KCH = 512           # xk stream chunk
NCH = KSH // KCH    # 2 xk chunks
F32 = mybir.dt.float32
F32R = mybir.dt.float32r
BF16 = mybir.dt.bfloat16
SCALE = 1.0 / float(np.sqrt(DOUT))
NEG = -1.0e9
NWARM = 16          # PE warm-up matmuls
EXP = mybir.ActivationFunctionType.Exp

_NC_CACHE = {}


def _emit_score_chunk(nc, psS, mfull_sb, kqt, xq_g, slab, zeros_sb, g, kt):
    """Scores+exp for one (group, k-tile) chunk of S^T.

    kqt is unscaled; 1/sqrt(d) is folded into the exp's scale (the
    additive mask scales along, -1e9/32 still floors the exp).
    kt == 2g+1: first 256 query columns fully masked -> zero-fill,
    compute the second half only.  kt == 2g: diagonal, additive mask.
    """
    if kt == 2 * g + 1:
        ps = psS.tile([P, QG], F32, name="score_ps")
        ph = ps[:, H:]
        for io in range(KO):
            nc.tensor.matmul(
                ph, lhsT=kqt[:, io, kt * P : (kt + 1) * P],
                rhs=xq_g[:, io, H:],
                start=(io == 0), stop=(io == KO - 1),
            )
        nc.vector.tensor_copy(slab[:, kt, :H], zeros_sb[:, :H])
        nc.vector.tensor_tensor(
            slab[:, kt, H:], ph, mfull_sb[:, :H], mybir.AluOpType.add
        )
        nc.scalar.activation(slab[:, kt, H:], slab[:, kt, H:], EXP, scale=SCALE)
        return
    ps = psS.tile([P, QG], F32, name="score_ps")
    for io in range(KO):
        nc.tensor.matmul(
            ps, lhsT=kqt[:, io, kt * P : (kt + 1) * P], rhs=xq_g[:, io, :],
            start=(io == 0), stop=(io == KO - 1),
        )
    if kt == 2 * g:
        nc.vector.tensor_tensor(
            slab[:, kt, :], ps, mfull_sb, mybir.AluOpType.add
        )
        nc.scalar.activation(slab[:, kt, :], slab[:, kt, :], EXP, scale=SCALE)
    else:
        nc.scalar.activation(slab[:, kt, :], ps, EXP, scale=SCALE)


def _build_bass():
    nc = bacc.Bacc()
    # host-repacked layouts: every DMA slice is contiguous per partition row
    xq4 = nc.declare_dram_parameter("xq4", [P, NG, KO, QG], F32R, isOutput=False)
    xk4 = nc.declare_dram_parameter("xk4", [P, NCH, KO, KCH], F32R, isOutput=False)
    xv4 = nc.declare_dram_parameter("xv4", [P, KO, KSH], BF16, isOutput=False)
    wq4 = nc.declare_dram_parameter("wq4", [P, KO, DO, P], F32R, isOutput=False)
    wk4 = nc.declare_dram_parameter("wk4", [P, DO, KO, P], F32R, isOutput=False)
    wv4 = nc.declare_dram_parameter("wv4", [P, 2, KO, 512], BF16, isOutput=False)
    mfull = nc.declare_dram_parameter("mfull", [P, QG], BF16, isOutput=False)
    ohat = nc.declare_dram_parameter("ohat", [S, DOUT], BF16, isOutput=True)
    l_out = nc.declare_dram_parameter("l", [1, S], F32, isOutput=True)

    with tile.TileContext(nc) as tc:
        # ---- constants
        cpool_cm = tc.tile_pool(name="const", bufs=1)
        cpool = cpool_cm.__enter__()
        z32 = cpool.tile([P, QG], F32, name="z32")
        nc.vector.memset(z32, 0.0)
        zeros_sb = cpool.tile([P, QG], F32R, name="zeros")
        nc.vector.tensor_copy(zeros_sb, z32)
        ones32 = cpool.tile([P, 1], F32, name="ones32")
        nc.vector.memset(ones32, 1.0)
        ones_sb = cpool.tile([P, 1], F32R, name="ones")
        nc.vector.tensor_copy(ones_sb, ones32)
        mfull_sb = cpool.tile([P, QG], BF16, name="mfull_sb")

        # ---- PE warm-up through the initial DMA wait
        with tc.tile_pool(name="ps_warm", bufs=1, space="PSUM") as psW:
            pw = psW.tile([P, QG], F32, name="warm_ps")
            for _ in range(NWARM):
                nc.tensor.matmul(
                    pw, lhsT=zeros_sb[:, :P], rhs=zeros_sb,
                    start=True, stop=True,
                )

        with tc.tile_pool(name="persist", bufs=1) as persist:
            v_sb = persist.tile([P, NT, DOUT], F32R, name="v")
            kqt_sb = persist.tile([P, KO, KSH], F32R, name="kqt")

            # wk now; wv reuses this slot once the K phase retires
            wpool_cm = tc.tile_pool(name="w_pool", bufs=1)
            wpool = wpool_cm.__enter__()
            wk_sb = wpool.tile([P, KO, DOUT], F32R, name="wk_sb", tag="w")

            xvpool_cm = tc.tile_pool(name="xv_pool", bufs=1)
            xvpool = xvpool_cm.__enter__()
            xv_sb = xvpool.tile([P, KO, KSH], BF16, name="xv_sb")

            wqpool_cm = tc.tile_pool(name="wq_pool", bufs=3)
            wqpool = wqpool_cm.__enter__()

            ktpool_cm = tc.tile_pool(name="kt_pool", bufs=1)
            ktpool = ktpool_cm.__enter__()
            kt_sb = ktpool.tile([P, DO, KSH], F32R, name="kt")

            # early loads (scalar HWDGE queue): wk slices, first wq slices,
            # the mask.  wq streams in rotating it-slices.
            for s_ in range(DO):
                nc.scalar.dma_start(
                    out=wk_sb[:, :, s_ * P : (s_ + 1) * P], in_=wk4[:, s_, :, :]
                )
            wq_tiles = {}
            for it in range(3):
                wq_tiles[it] = wqpool.tile([P, DO, P], F32R, name="wq_sl")
                nc.scalar.dma_start(out=wq_tiles[it][:, :, :], in_=wq4[:, it, :, :])
            nc.scalar.dma_start(out=mfull_sb, in_=mfull[:, :])

            # one PSUM pool spans K/KQ/V: tag rotation, no boundary syncs
            pspool_cm = tc.tile_pool(name="ps_main", bufs=4, space="PSUM")
            pspool = pspool_cm.__enter__()

            # ---- Phase K: K^T = WK^T @ Xk^T
            with tc.tile_pool(name="xs_k", bufs=2) as xpool:
                for c in range(NCH):
                    x_sb = xpool.tile([P, KO, KCH], F32R, name="xk_chunk")
                    nc.sync.dma_start(out=x_sb[:, :, :], in_=xk4[:, c, :, :])
                    for o in range(DO):
                        ps = pspool.tile([P, QG], F32, name="mm_ps")
                        for k in range(KO):
                            nc.tensor.matmul(
                                ps,
                                lhsT=wk_sb[:, k, o * P : (o + 1) * P],
                                rhs=x_sb[:, k, :],
                                start=(k == 0),
                                stop=(k == KO - 1),
                            )
                        nc.vector.tensor_copy(
                            kt_sb[:, o, c * KCH : (c + 1) * KCH], ps
                        )

            # post-K loads: wv into wk's slot (fires at K retire), xv behind
            # the xk stream on sync (single contiguous 2MB transfer).
            wv_sb = wpool.tile([P, KO, DOUT], BF16, name="wv_sb", tag="w")
            for dh in range(2):
                nc.scalar.dma_start(
                    out=wv_sb[:, :, dh * 512 : (dh + 1) * 512],
                    in_=wv4[:, dh, :, :],
                )
            nc.sync.dma_start(out=xv_sb[:, :, :], in_=xv4[:, :, :])

            # ---- Phase KQ: KQ^T = WQ @ K^T (unscaled)
            for it in range(KO):
                wq_t = wq_tiles[it]
                for kc in range(KSH // 512):
                    ps = pspool.tile([P, QG], F32, name="mm_ps")
                    for do in range(DO):
                        nc.tensor.matmul(
                            ps,
                            lhsT=wq_t[:, do, :],
                            rhs=kt_sb[:, do, kc * 512 : (kc + 1) * 512],
                            start=(do == 0),
                            stop=(do == DO - 1),
                        )
                    nc.vector.tensor_copy(
                        kqt_sb[:, it, kc * 512 : (kc + 1) * 512], ps
                    )
                if it + 3 < KO:
                    wq_tiles[it + 3] = wqpool.tile([P, DO, P], F32R, name="wq_sl")
                    nc.scalar.dma_start(
                        out=wq_tiles[it + 3][:, :, :], in_=wq4[:, it + 3, :, :]
                    )

            ktpool_cm.__exit__(None, None, None)

            # ---- attention pools + early query-group prefetches
            xqpool_cm = tc.tile_pool(name="xq_pool", bufs=2)
            xqpool = xqpool_cm.__enter__()
            slabpool_cm = tc.tile_pool(name="slab", bufs=2)
            slabpool = slabpool_cm.__enter__()
            accpool_cm = tc.tile_pool(name="l_acc", bufs=2)
            accpool = accpool_cm.__enter__()
            lpool_cm = tc.tile_pool(name="l_row", bufs=2)
            lpool = lpool_cm.__enter__()
            opool_cm = tc.tile_pool(name="o_out", bufs=3)
            opool = opool_cm.__enter__()

            xq_tiles = {}
            xq_tiles[0] = xqpool.tile([P, KO, QG], F32R, name="xq_group")
            nc.scalar.dma_start(out=xq_tiles[0][:, :, :], in_=xq4[:, 0, :, :])
            xq_tiles[1] = xqpool.tile([P, KO, QG], F32R, name="xq_group")
            nc.sync.dma_start(out=xq_tiles[1][:, :, :], in_=xq4[:, 1, :, :])

            # ---- Phase V: V = Xv @ WV (bf16 in, fp32 accum)
            for t in range(NT):
                for dh in range(DOUT // 512):
                    ps = pspool.tile([P, QG], F32, name="mm_ps")
                    for k in range(KO):
                        nc.tensor.matmul(
                            ps,
                            lhsT=xv_sb[:, k, t * P : (t + 1) * P],
                            rhs=wv_sb[:, k, dh * 512 : (dh + 1) * 512],
                            start=(k == 0),
                            stop=(k == KO - 1),
                        )
                    nc.vector.tensor_copy(
                        v_sb[:, t, dh * 512 : (dh + 1) * 512], ps
                    )

            pspool_cm.__exit__(None, None, None)

            # ---- Attention, software-pipelined ascending groups:
            # S0 S1 l0 O0 S2 l1 O1 S3 l2 O2 l3 O3
            with (
                tc.tile_pool(name="ps_s", bufs=3, space="PSUM") as psS,
                tc.tile_pool(name="ps_l", bufs=1, space="PSUM") as psL,
                tc.tile_pool(name="ps_o", bufs=3, space="PSUM") as psO,
            ):
                slabs = {}

                def emit_scores(g):
                    lim = min(NT, 2 * g + 2)
                    slab = slabpool.tile([P, NT, QG], F32R, name="expT")
                    slabs[g] = slab
                    for kt in range(lim):
                        _emit_score_chunk(
                            nc, psS, mfull_sb, kqt_sb, xq_tiles[g],
                            slab, zeros_sb, g, kt,
                        )
                    if g + 2 < NG:
                        nxt = xqpool.tile([P, KO, QG], F32R, name="xq_group")
                        xq_tiles[g + 2] = nxt
                        nc.sync.dma_start(
                            out=nxt[:, :, :], in_=xq4[:, g + 2, :, :]
                        )

                def emit_l(g):
                    lim = min(NT, 2 * g + 2)
                    slab = slabs[g]
                    acc = accpool.tile([P, QG], F32R, name="acc")
                    nc.vector.tensor_tensor(
                        acc, slab[:, 0, :], slab[:, 1, :], mybir.AluOpType.add
                    )
                    for kt in range(2, lim):
                        nc.vector.tensor_tensor(
                            acc, acc, slab[:, kt, :], mybir.AluOpType.add
                        )
                    ps_l = psL.tile([1, QG], F32, name="l_ps")
                    nc.tensor.matmul(
                        ps_l, lhsT=ones_sb, rhs=acc, start=True, stop=True
                    )
                    l_t = lpool.tile([1, QG], F32, name="l_t")
                    nc.vector.tensor_copy(l_t, ps_l)
                    nc.scalar.dma_start(
                        out=l_out[:, g * QG : (g + 1) * QG], in_=l_t
                    )

                def emit_out(g):
                    lim = min(NT, 2 * g + 2)
                    slab = slabs[g]
                    for t in range(QG // P):
                        kts = (
                            list(range(lim - 1)) if t < 2 else list(range(lim))
                        )
                        q0 = g * QG + t * P
                        last_tile = g == NG - 1 and t == QG // P - 1
                        o_sb = opool.tile([P, DOUT], BF16, name="attn_out")
                        for dh in range(DOUT // 512):
                            ps = psO.tile([P, 512], F32, name="out_ps")
                            for kt in kts:
                                nc.tensor.matmul(
                                    ps,
                                    lhsT=slab[:, kt, t * P : (t + 1) * P],
                                    rhs=v_sb[:, kt, dh * 512 : (dh + 1) * 512],
                                    start=(kt == kts[0]),
                                    stop=(kt == kts[-1]),
                                )
                            nc.scalar.copy(
                                o_sb[:, dh * 512 : (dh + 1) * 512], ps
                            )
                            if last_tile:
                                nc.sync.dma_start(
                                    out=ohat[
                                        q0 : q0 + P, dh * 512 : (dh + 1) * 512
                                    ],
                                    in_=o_sb[:, dh * 512 : (dh + 1) * 512],
                                )
                        if not last_tile:
                            nc.sync.dma_start(
                                out=ohat[q0 : q0 + P, :], in_=o_sb
                            )

                emit_scores(0)
                emit_scores(1)
                emit_l(0)
                emit_out(0)
                emit_scores(2)
                emit_l(1)
                emit_out(1)
                emit_scores(3)
                emit_l(2)
                emit_out(2)
                emit_l(3)
                emit_out(3)

            opool_cm.__exit__(None, None, None)
            lpool_cm.__exit__(None, None, None)
            accpool_cm.__exit__(None, None, None)
            slabpool_cm.__exit__(None, None, None)
            xqpool_cm.__exit__(None, None, None)
            wqpool_cm.__exit__(None, None, None)
            xvpool_cm.__exit__(None, None, None)
            wpool_cm.__exit__(None, None, None)
        cpool_cm.__exit__(None, None, None)
    nc.finalize()
    return nc


def _get_nc():
    if "nc" not in _NC_CACHE:
        _NC_CACHE["nc"] = _build_bass()
    return _NC_CACHE["nc"]


def _key_index(hk):
    """Global key rows owned by core hk: interleaved 128-row blocks."""
    blocks = np.arange(hk, S // P, 2)
    return (blocks[:, None] * P + np.arange(P)[None, :]).reshape(-1)


def _mask_full(hk):
    """Group-invariant additive causal mask for the diagonal chunks.

    Chunk kt == 2g covers global keys (4g+hk)*128 + p against queries
    512g + j: causal k > q reduces to hk*128 + p > j for every g; chunk
    kt == 2g+1's live half reduces to this tile's first 256 columns.
    """
    p = np.arange(P)[:, None]
    j = np.arange(QG)[None, :]
    m = np.where(hk * P + p > j, np.float32(NEG), np.float32(0.0))
    return np.ascontiguousarray(m.astype(ml_dtypes.bfloat16))


def kernel(
    inputs_for_keys,
    inputs_for_values,
    inputs_for_queries,
    WK,
    WV,
    WQ,
    _trace=False,
):
    xk = np.asarray(inputs_for_keys, dtype=np.float32)
    xv = np.asarray(inputs_for_values, dtype=np.float32)
    xq = np.asarray(inputs_for_queries, dtype=np.float32)
    wk_ = np.asarray(WK, dtype=np.float32)
    wv_ = np.asarray(WV, dtype=np.float32)
    wq_ = np.asarray(WQ, dtype=np.float32)

    # host repack: [128, slice, ...] with contiguous per-partition rows
    wk4 = np.ascontiguousarray(
        wk_.reshape(KO, P, DO, P).transpose(1, 2, 0, 3)
    )
    wq4 = np.ascontiguousarray(
        wq_.T.reshape(DO, P, KO, P).transpose(1, 2, 0, 3)
    )
    wv4 = np.ascontiguousarray(
        wv_.astype(ml_dtypes.bfloat16).reshape(KO, P, 2, 512).transpose(1, 2, 0, 3)
    )

    kidx = {hk: _key_index(hk) for hk in (0, 1)}
    masks = {hk: _mask_full(hk) for hk in (0, 1)}
    xq4b = [
        np.ascontiguousarray(
            xq[b].T.reshape(KO, P, NG, QG).transpose(1, 2, 0, 3)
        )
        for b in range(B)
    ]

    in_maps = []
    for i in range(8):
        b, hk = i // 2, i % 2
        xkT = xk[b][kidx[hk]].T      # [DIN, KSH]
        xvT = xv[b][kidx[hk]].T
        in_maps.append(
            {
                "xq4": xq4b[b],
                "xk4": np.ascontiguousarray(
                    xkT.reshape(KO, P, NCH, KCH).transpose(1, 2, 0, 3)
                ),
                "xv4": np.ascontiguousarray(
                    xvT.astype(ml_dtypes.bfloat16).reshape(KO, P, KSH).transpose(1, 0, 2)
                ),
                "wq4": wq4,
                "wk4": wk4,
                "wv4": wv4,
                "mfull": masks[hk],
            }
        )

    nc = _get_nc()
    res = run_bass_kernel_spmd(nc, in_maps, list(range(8)), trace=_trace)

    out = np.empty((B, S, DOUT), dtype=np.float32)
    for b in range(B):
        r0 = res.results[2 * b]
        r1 = res.results[2 * b + 1]
        den = (r0["l"] + r1["l"]).reshape(S, 1)
        out[b] = (
            r0["ohat"].astype(np.float32) + r1["ohat"].astype(np.float32)
        ) / den
    if _trace:
        return out, res
    return out
